# revision 61
# baseline (speedup 1.0000x reference)
import sys

sys.path.insert(0, "/opt/trn_rl_repo")

import numpy as np
import os as _os

# Problem constants (hardcoded per harness contract)
B = 64          # full batch
NC_CORES = 8
BPC = 8         # batches per core
N = 1024
D = 768
NS = 16         # n_slots
KT = 8          # n-tiles of 128
DT = 6          # d-tiles of 128
NG = 8          # column groups per batch in blocked layout (N / 128)
BG = BPC * NG   # 64 partitions of blocked loop state (legacy)
NGRP = 2        # phase-B groups (each 4 batches) for A/B overlap
BPG = BPC // NGRP
PG = BPG * NG   # 32 partitions of blocked loop state per group

REPS = int(_os.environ.get("KERNEL_REPS", "1"))
# phase gating for timing attribution: "A", "AB", or "ABC" (full kernel)
PHASES = _os.environ.get("KERNEL_PHASES", "ABC")

_CACHE = {}


def _build_nc(debug=False, reps=None):
    import concourse.bacc as bacc
    import concourse.tile as tile
    import concourse.mybir as mybir

    if reps is None:
        reps = REPS

    fp32 = mybir.dt.float32

    nc = bacc.Bacc(
        "TRN2",
        target_bir_lowering=False,
        debug=False,
        enable_asserts=False,
        num_devices=NC_CORES,
    )

    fp16 = mybir.dt.float16

    f_dr = nc.dram_tensor("features", [BPC, N, D], fp32, kind="ExternalInput").ap()
    ident_dr = nc.dram_tensor("identity", [128, 128], fp32, kind="ExternalInput").ap()
    idh_dr = nc.dram_tensor("identh", [128, 128], fp16, kind="ExternalInput").ap()
    ec_dr = nc.dram_tensor("econst", [BPG, PG], fp32, kind="ExternalInput").ap()
    pc_dr = nc.dram_tensor("pcol", [NGRP, PG, 2], fp32, kind="ExternalInput").ap()
    cr_dr = nc.dram_tensor("crow", [1, 128], fp32, kind="ExternalInput").ap()
    out_dr = nc.dram_tensor("slots", [BPC, NS, D], fp32, kind="ExternalOutput").ap()
    # Raw-Gram scratch (fp32): rows (b*N + n)*NG + g of 128 cols; the greedy
    # loop gathers row (code*8 + g) per (b,g) partition and normalizes with
    # the gathered 1/||f|| scalars (rsal_dr).
    g_dr = nc.dram_tensor("g_scratch", [BPC * N * NG, 128], fp32, kind="Internal").ap()
    g_wr = g_dr.rearrange("(r e) c -> r (e c)", e=NG)  # [BPC*N, N] fp32
    # per-position (-1/||f||, 1/||f||, ||f||/2) rows for phase-B gathers
    rs_dr = nc.dram_tensor("rsal_scratch", [BPC * N, 3], fp32, kind="Internal").ap()

    with tile.TileContext(nc) as tc:
        with (
            tc.tile_pool(name="main", bufs=1) as mp,
            tc.tile_pool(name="fbuf", bufs=2) as fbp,
            tc.tile_pool(name="sqscr", bufs=1) as sqp,
            tc.tile_pool(name="fnt", bufs=1) as ftp,
            tc.tile_pool(name="gst", bufs=2) as gsp,
            tc.tile_pool(name="small", bufs=2) as smp,
            tc.tile_pool(name="psT", bufs=2, space="PSUM") as ppT,
            tc.tile_pool(name="psG", bufs=2, space="PSUM") as ppG,
            tc.tile_pool(name="psB", bufs=1, space="PSUM") as ppB,
        ):
            ident = mp.tile([128, 128], fp32)
            nc.sync.dma_start(ident, ident_dr)
            identh = mp.tile([128, 128], fp16)
            nc.sync.dma_start(identh, idh_dr)
            E_sb = mp.tile([BPG, PG], fp32)
            nc.sync.dma_start(E_sb, ec_dr)
            pcol = []
            for g in range(NGRP):
                pc = mp.tile([PG, 2], fp32, name=f"pcol{g}")
                nc.sync.dma_start(pc, pc_dr[g])
                pcol.append(pc)
            crow = mp.tile([1, 128], fp32)
            nc.sync.dma_start(crow, cr_dr)

            consts = (ident, identh, E_sb, pcol, crow)
            for _rep in range(reps):
                _run_once(nc, tc, tile, mybir,
                          mp, fbp, ftp, gsp, smp, sqp, ppT, ppG, ppB,
                          consts, f_dr, out_dr, g_dr, g_wr, rs_dr)
                if reps > 1:
                    tc.strict_bb_all_engine_barrier()

    nc.compile()
    return nc


def _run_once(nc, tc, tile, mybir,
              mp, fbp, ftp, gsp, smp, sqp, ppT, ppG, ppB,
              consts, f_dr, out_dr, g_dr, g_wr, rs_dr):
    from concourse.bass import IndirectOffsetOnAxis
    from concourse.tile_rust import add_dep_helper

    fp32 = mybir.dt.float32
    fp32r = mybir.dt.float32r
    fp16 = mybir.dt.float16
    i32 = mybir.dt.int32
    u32 = mybir.dt.uint32
    u8 = mybir.dt.uint8
    Alu = mybir.AluOpType
    Act = mybir.ActivationFunctionType
    X = mybir.AxisListType.X

    ident, identh, E_sb, pcol, crow = consts

    # persistent tiles
    pk = [mp.tile([128, BPG, KT], fp32, name=f"pk{g}") for g in range(NGRP)]
    pk2 = [mp.tile([128, BPG, KT], fp32, name=f"pk2{g}") for g in range(NGRP)]
    wT = [mp.tile([128, KT, BPG, NS], fp32r, name=f"wT{g}") for g in range(NGRP)]
    wsum = [mp.tile([PG, NS], fp32, name=f"wsum{g}") for g in range(NGRP)]

    # ---------------- Phase A: per-batch RAW Gram -----------------------
    # The Gram is computed on raw features (fp32r inputs, 1 cyc/row);
    # normalization scalars 1/||f|| are applied in phase B from rmT (per
    # column) and gathered (-r, r, sal/2) rows (per selected position).
    wdeps = [[], []]   # gram-write DMA instrs per group (for gather RAW deps)
    rdeps = [[], []]   # rsal-write DMA instrs per group

    def emit_batch_a(b):
        grp = b // BPG
        bl = b % BPG
        sal2 = smp.tile([128, KT], fp32, tag="sal2")
        fview = f_dr[b].rearrange("(kt p) d -> p kt d", p=128)
        fT = ftp.tile([128, DT, N], fp32r, tag="fT")
        for h in range(2):
            f_sb = fbp.tile([128, 4, D], fp32, tag="f")
            nc.sync.dma_start(f_sb, fview[:, 4 * h:4 * h + 4])
            for k in range(4):
                kt = 4 * h + k
                if k % 2 == 0:
                    sq = sqp.tile([128, D], fp32, tag="sq")
                    nc.scalar.activation(
                        sq, f_sb[:, k], Act.Square,
                        accum_out=sal2[:, kt:kt + 1],
                    )
                else:
                    sqd = sqp.tile([128, D], fp32, tag="sqd")
                    nc.vector.tensor_tensor_reduce(
                        sqd, f_sb[:, k], f_sb[:, k], 1.0, 0.0,
                        Alu.mult, Alu.add, sal2[:, kt:kt + 1],
                    )
                # raw transpose (identity): fT[d, n] = f[n, d]
                for dt0, ndt in ((0, 4), (4, 2)):
                    tp = ppT.tile([128, 512], fp32, tag="tpk")
                    for j in range(ndt):
                        nc.tensor.transpose(
                            tp[:, j * 128:(j + 1) * 128],
                            f_sb[:, k, (dt0 + j) * 128:(dt0 + j + 1) * 128],
                            ident,
                        )
                    src = tp[:, 0:ndt * 128].rearrange("p (a c) -> p a c", c=128)
                    dst = fT[:, dt0:dt0 + ndt, kt * 128:(kt + 1) * 128]
                    if (kt + dt0) % 2 == 0:
                        nc.scalar.copy(dst, src)
                    else:
                        nc.vector.tensor_copy(dst, src)

        # saliency + reciprocal; (-r, r, sal/2) rows for phase-B gathers
        nc.scalar.activation(pk[grp][:, bl], sal2, Act.Sqrt)
        nc.vector.reciprocal(pk2[grp][:, bl], pk[grp][:, bl])
        nr = smp.tile([128, KT, 3], fp32, tag="nr")
        nc.vector.tensor_scalar(
            nr[:, :, 0], pk2[grp][:, bl], -1.0, None, op0=Alu.mult
        )
        nc.vector.tensor_copy(nr[:, :, 1], pk2[grp][:, bl])
        nc.vector.tensor_scalar(
            nr[:, :, 2], pk[grp][:, bl], 0.5, None, op0=Alu.mult
        )
        rdma = nc.sync.dma_start(
            rs_dr[b * N:(b + 1) * N].rearrange("(kt p) c -> p kt c", p=128),
            nr,
        )
        rdeps[grp].append(rdma)

        # G_raw = f @ f.T upper triangle in 256-wide chunks; row-pairs share
        # one gstage tile so each pair is written with a single DMA
        gst_tiles = []
        for rp in range(4):
            w = N - rp * 256
            gst_tiles.append(gsp.tile([128, 2, w], fp32, tag=f"rp{rp}",
                                      name=f"rp{rp}"))
        for i in range(KT):
            rp = i // 2
            c0 = rp * 256
            for jc in range(rp, 4):
                gp = ppG.tile([128, 256], fp32, tag="gps")
                for dt in range(DT):
                    nc.tensor.matmul(
                        gp,
                        fT[:, dt, i * 128:(i + 1) * 128],
                        fT[:, dt, jc * 256:(jc + 1) * 256],
                        start=(dt == 0),
                        stop=(dt == DT - 1),
                    )
                co = jc * 256 - c0
                if (i + jc) % 2 == 0:
                    nc.scalar.copy(gst_tiles[rp][:, i % 2, co:co + 256], gp)
                else:
                    nc.vector.tensor_copy(gst_tiles[rp][:, i % 2, co:co + 256], gp)
            if i % 2 == 1:
                wdma = nc.sync.dma_start(
                    g_wr[b * N + (i - 1) * 128: b * N + (i + 1) * 128, c0:]
                    .rearrange("(two p) w -> p two w", p=128),
                    gst_tiles[rp],
                )
                wdeps[grp].append(wdma)

        # mirror blocks below the diagonal: row-block j cols [0, (j//2)*256),
        # assembled per row-pair so each pair is one DMA; mirror packs share
        # the transpose-pack PSUM tag to stay within 8 banks
        for jp in range(1, 4):
            nmb = jp
            mst = gsp.tile([128, 2, nmb * 256], fp32, tag="mst")
            for jj in range(2):
                j = 2 * jp + jj
                for k in range(nmb):
                    mp_ps = ppT.tile([128, 256], fp32, tag="tpk")
                    so = j * 128 - k * 256
                    for half in range(2):
                        nc.tensor.transpose(
                            mp_ps[:, half * 128:(half + 1) * 128],
                            gst_tiles[k][:, half, so:so + 128],
                            ident,
                        )
                    if (j + k) % 2 == 0:
                        nc.scalar.copy(mst[:, jj, k * 256:(k + 1) * 256], mp_ps)
                    else:
                        nc.vector.tensor_copy(
                            mst[:, jj, k * 256:(k + 1) * 256], mp_ps)
            wdma = nc.sync.dma_start(
                g_wr[b * N + 2 * jp * 128: b * N + (2 * jp + 2) * 128,
                     0:nmb * 256]
                .rearrange("(two p) w -> p two w", p=128),
                mst,
            )
            wdeps[grp].append(wdma)

    # ---------------- Phase B: 16-step greedy loop, 2 groups ------------
    # Emission order is A(0-3), B(g0), A(4-7), B(g1), C: the scheduler's
    # stream order follows emission, so B(g0)'s latency-bound chain
    # interleaves into A(4-7)'s dense engine work.
    mask, msal, rmT = [], [], []
    sims, us, t1s, rsps = [], [], [], []
    mx8, ix8, vi, viT = [], [], [], []
    bmax, eq, code, nst, nT, offs, offs2 = [], [], [], [], [], [], []
    gate, w1, aggw, aggw2, vtmp = [], [], [], [], []

    def emit_group_b_init(g):
        salT_ps = ppB.tile([PG, 128], fp32, tag=f"tps{g}")
        nc.tensor.transpose(
            salT_ps, pk[g].rearrange("p b k -> p (b k)"), ident
        )
        ms = mp.tile([PG, 128], fp32, name=f"msal{g}")
        nc.scalar.copy(ms, salT_ps)
        msal.append(ms)
        rmT_ps = ppB.tile([PG, 128], fp32, tag=f"tps{g}")
        nc.tensor.transpose(
            rmT_ps, pk2[g].rearrange("p b k -> p (b k)"), ident
        )
        rt = mp.tile([PG, 128], fp32, name=f"rmT{g}")
        nc.scalar.copy(rt, rmT_ps)
        rmT.append(rt)
        mk = mp.tile([PG, 128], fp32, name=f"mask{g}")
        nc.vector.memset(mk, 1.0)
        mask.append(mk)
        sims.append([mp.tile([PG, 128], fp32, name=f"sim{g}_{i}")
                     for i in range(2)])
        us.append([mp.tile([PG, 128], fp32, name=f"u{g}_{i}")
                   for i in range(2)])
        t1s.append([mp.tile([PG, 128], fp32, name=f"t1{g}_{i}")
                    for i in range(2)])
        rsps.append([mp.tile([PG, 3], fp32, name=f"rsp{g}_{i}")
                     for i in range(2)])
        mx8.append(mp.tile([PG, 8], fp32, name=f"mx8{g}"))
        ix8.append(mp.tile([PG, 8], u32, name=f"ix8{g}"))
        vi.append(mp.tile([PG, 1], fp32, name=f"vi{g}"))
        viT.append(mp.tile([1, 2 * PG], fp32, name=f"viT{g}"))
        bmax.append(mp.tile([1, BPG], fp32, name=f"bmax{g}"))
        eq.append(mp.tile([1, PG], u8, name=f"eq{g}"))
        code.append(mp.tile([1, PG], fp32, name=f"code{g}"))
        nst.append(mp.tile([1, BPG], fp32, name=f"nst{g}"))
        nT.append(mp.tile([BPG, 1], fp32, name=f"nT{g}"))
        offs.append(mp.tile([PG, 1], i32, name=f"offs{g}"))
        offs2.append(mp.tile([PG, 1], i32, name=f"offs2{g}"))
        gate.append(mp.tile([PG, 128], fp32, name=f"gate{g}"))
        w1.append(mp.tile([PG, 128], fp32, name=f"w1{g}"))
        aggw.append(mp.tile([PG, 128], fp32, name=f"aggw{g}"))
        aggw2.append(mp.tile([PG, 128], fp32, name=f"aggw2{g}"))
        vtmp.append(mp.tile([PG, 128], fp32, name=f"vtmp{g}"))

    def emit_deferred(g, t):
        # off-critical aggregation + mask update for step t of group g
        u = us[g][t % 2]
        t1 = t1s[g][t % 2]
        rsp = rsps[g][t % 2]
        # w1 = sim*mask = (t1 * r_sel) * mask, fused on DVE
        nc.vector.scalar_tensor_tensor(
            w1[g], t1, rsp[:, 1:2], mask[g], op0=Alu.mult, op1=Alu.mult
        )
        # gate: sim > 0.5  <=>  t1 > 0.5*sal_sel
        nc.vector.tensor_scalar(gate[g], t1, rsp[:, 2:3], None, op0=Alu.is_gt)
        nc.vector.tensor_mul(aggw[g], w1[g], gate[g])
        nc.scalar.activation(
            aggw2[g], aggw[g], Act.Copy, accum_out=wsum[g][:, t:t + 1]
        )
        awT_ps = ppG.tile([128, PG], fp32, tag="awt")
        nc.tensor.transpose(awT_ps, aggw[g], ident[:PG, :PG])
        nc.scalar.copy(
            wT[g][:, :, :, t],
            awT_ps.rearrange("p (b gg) -> p gg b", b=BPG),
        )
        # mask *= clamp(1 - sim, 0, 1)  (u already clamped)
        nc.vector.tensor_mul(mask[g], mask[g], u)

    def emit_step(g, t):
        s = sims[g][t % 2]
        u = us[g][t % 2]
        nc.vector.max(out=mx8[g], in_=msal[g])
        nc.vector.max_index(out=ix8[g], in_max=mx8[g], in_values=msal[g])
        # local idx -> global code b*1024 + g2*128 + c
        nc.vector.tensor_scalar(
            vi[g], ix8[g][:, 0:1], pcol[g][:, 0:1], None, op0=Alu.add
        )
        vi2_ps = ppB.tile([1, 2 * PG], fp32, tag=f"tps{g}")
        nc.tensor.transpose(vi2_ps[:, 0:PG], mx8[g][:, 0:1], ident[:PG, :PG])
        nc.tensor.transpose(vi2_ps[:, PG:2 * PG], vi[g], ident[:PG, :PG])
        nc.vector.tensor_copy(viT[g], vi2_ps)
        valT = viT[g][:, 0:PG]
        idxT = viT[g][:, PG:2 * PG]
        # per-batch max over groups, first-index tiebreak via min-code
        nc.vector.tensor_reduce(
            bmax[g], valT.rearrange("o (b gg) -> o b gg", b=BPG),
            axis=X, op=Alu.max,
        )
        nc.vector.tensor_tensor(
            eq[g].rearrange("o (b gg) -> o b gg", b=BPG),
            valT.rearrange("o (b gg) -> o b gg", b=BPG),
            bmax[g].unsqueeze(2).to_broadcast([1, BPG, NG]),
            op=Alu.is_ge,
        )
        nc.vector.select(code[g], eq[g], idxT, crow[0:1, 0:PG])
        nc.vector.tensor_reduce(
            nst[g], code[g].rearrange("o (b gg) -> o b gg", b=BPG),
            axis=X, op=Alu.min,
        )
        nT_ps = ppB.tile([BPG, 1], fp32, tag=f"tps{g}")
        nc.tensor.transpose(nT_ps, nst[g], ident[:1, :1])
        nc.vector.tensor_copy(nT[g], nT_ps)
        rep_ps = ppB.tile([PG, 1], fp32, tag=f"tps{g}")
        nc.tensor.matmul(rep_ps, E_sb, nT[g], start=True, stop=True)
        nc.vector.tensor_scalar(
            offs2[g], rep_ps, 1.0, None, op0=Alu.mult
        )
        nc.vector.tensor_scalar(
            offs[g], rep_ps, 8.0, pcol[g][:, 1:2], op0=Alu.mult, op1=Alu.add
        )
        rsp = rsps[g][t % 2]
        gi = nc.gpsimd.indirect_dma_start(
            out=s,
            out_offset=None,
            in_=g_dr,
            in_offset=IndirectOffsetOnAxis(ap=offs[g], axis=0),
        )
        gi2 = nc.gpsimd.indirect_dma_start(
            out=rsp,
            out_offset=None,
            in_=rs_dr,
            in_offset=IndirectOffsetOnAxis(ap=offs2[g], axis=0),
        )
        if t == 0 and not _os.environ.get("KERNEL_NO_GATHER_DEPS"):
            for wdma in wdeps[g]:
                add_dep_helper(gi.ins, wdma.ins, sync=True,
                               reason="gram writes visible before gather")
            for rdma in rdeps[g]:
                add_dep_helper(gi2.ins, rdma.ins, sync=True,
                               reason="rsal writes visible before gather")
        if t > 0:
            emit_deferred(g, t - 1)
        # critical tail: sim = raw*r_m*r_sel;
        # msal *= clamp(1 - sim, 0, 1), all on DVE (no engine hops)
        t1 = t1s[g][t % 2]
        nc.vector.tensor_mul(t1, s, rmT[g])
        nc.vector.tensor_scalar(
            vtmp[g], t1, rsp[:, 0:1], 1.0, op0=Alu.mult, op1=Alu.add
        )
        nc.vector.tensor_scalar(
            u, vtmp[g], 0.0, 1.0, op0=Alu.max, op1=Alu.min
        )
        nc.vector.tensor_mul(msal[g], msal[g], u)

    def emit_group_b(g):
        emit_group_b_init(g)
        for t in range(NS):
            emit_step(g, t)
        emit_deferred(g, NS - 1)

    # interleaved emission: A(0-3), B(g0), A(4-7), B(g1)
    for b in range(BPG):
        emit_batch_a(b)
    if "B" in PHASES:
        emit_group_b(0)
    for b in range(BPG, BPC):
        emit_batch_a(b)
    if "B" not in PHASES:
        return
    emit_group_b(1)

    if "C" not in PHASES:
        return

    # ---------------- Phase C: slot matmuls -------------------------
    recip = []
    for g in range(NGRP):
        wsT_ps = ppB.tile([NS, PG], fp32, tag=f"tps{g}")
        nc.tensor.transpose(wsT_ps, wsum[g], ident[:PG, :PG])
        wsT = smp.tile([NS, PG], fp32, tag="wsT")
        nc.scalar.copy(wsT, wsT_ps)
        wsum_b = smp.tile([NS, BPG], fp32, tag="wsb")
        nc.vector.tensor_reduce(
            wsum_b, wsT.rearrange("p (b gg) -> p b gg", b=BPG),
            axis=X, op=Alu.add,
        )
        nc.vector.tensor_scalar(wsum_b, wsum_b, 1e-8, None, op0=Alu.add)
        rc = mp.tile([NS, BPG], fp32, name=f"recip{g}")
        nc.vector.reciprocal(rc, wsum_b)
        recip.append(rc)

    for b in range(BPC):
        g = b // BPG
        bl = b % BPG
        f_c = fbp.tile([128, KT, D], fp32, tag="fc")
        nc.sync.dma_start(f_c, f_dr[b].rearrange("(kt p) d -> p kt d", p=128))
        slot_sb = sqp.tile([NS, D], fp32, tag="slot")
        for h0, h1 in ((0, 512), (512, D)):
            sp = ppT.tile([NS, h1 - h0], fp32, tag="tpk")
            for kt in range(KT):
                nc.tensor.matmul(
                    sp,
                    wT[g][:, kt, bl, :],
                    f_c[:, kt, h0:h1].bitcast(fp32r),
                    start=(kt == 0),
                    stop=(kt == KT - 1),
                )
            nc.scalar.activation(
                slot_sb[:, h0:h1], sp, Act.Copy,
                scale=recip[g][:, bl:bl + 1]
            )
        nc.sync.dma_start(out_dr[b], slot_sb)


def _get_nc(debug=False, reps=None):
    key = ("nc", debug, reps if reps is not None else REPS, PHASES)
    if key not in _CACHE:
        _CACHE[key] = _build_nc(debug, reps=reps)
    return _CACHE[key]


def _consts():
    ident = np.eye(128, dtype=np.float32)
    identh = np.eye(128, dtype=np.float16)
    E = np.zeros((BPG, PG), dtype=np.float32)
    for p in range(PG):
        E[p // NG, p] = 1.0
    pcol = np.zeros((NGRP, PG, 2), dtype=np.float32)
    for g in range(NGRP):
        for p in range(PG):
            b = g * BPG + p // NG
            pcol[g, p, 0] = b * N + (p % NG) * 128
            pcol[g, p, 1] = p % NG
    crow = np.zeros((1, 128), dtype=np.float32)
    crow[0, :PG] = 65536.0                      # BIG (> any code)
    return ident, identh, E, pcol, crow


def _make_in_maps(feats):
    ident, identh, E, pcol, crow = _consts()
    return [
        {
            "features": feats[i * BPC:(i + 1) * BPC],
            "identity": ident,
            "identh": identh,
            "econst": E,
            "pcol": pcol,
            "crow": crow,
        }
        for i in range(NC_CORES)
    ]


def kernel(features, batch_size=None, **_kw):
    from concourse import bass_utils

    nc = _get_nc(reps=1)
    feats = np.ascontiguousarray(np.asarray(features, dtype=np.float32))
    in_maps = _make_in_maps(feats)
    res = bass_utils.run_bass_kernel_spmd(
        nc, in_maps, core_ids=list(range(NC_CORES))
    )
    outs = [np.asarray(res.results[i]["slots"]) for i in range(NC_CORES)]
    return np.concatenate(outs, axis=0).astype(np.float32)
